# revision 10
# baseline (speedup 1.0000x reference)
"""CopyGenerator kernel for Trainium2 (Bass/Tile), vocab-parallel over 8 cores.

Per core c (vocab shard [c*4000, (c+1)*4000)):
  A tiny dummy AllGather fires at t=0 to absorb the cross-core rendezvous
  barrier while compute proceeds.
  Attention for ALL 8 batches is computed locally (redundantly) on every
  core — cheaper than an AllGather whose latency floor + launch skew was
  ~60us.  q/k projections are batched into N=1024 matmuls.
  pass 1: gen = htgt @ emb_shard.T  (PE, fp8 DoubleRow: emb pre-scaled by
    64 and htgt by 8 on the host into fp8 normal range; exp's free scale
    1/512 undoes it).  e = exp(gen/512) with fused row-sum accumulation.
    Z = allreduce_add(rowsum), two batch groups, first AR overlaps the
    second half of pass 1.
  pass 2: psum = K*copy_p + diag(K*(1-a)/a) @ e, all on the PE (attnT @
    K*onehot(src) accumulated with the diag matmul).  The 1/Z lives only
    in the LN scale a/(K*Z), so the PE pipeline never waits on the
    AllReduce — only the final LN does.
      check: (a/(KZ))*(K*cp + K*(1-a)/a*e) = a*cp + (1-a)*e/Z. ✓

All transposed operands are pre-transposed/cast on the host and DMA
straight into SBUF (no PE transposes except the 128x128 attn transpose).
The vocab shard is zero-padded to 4096 for PSUM bank alignment; pad
columns add exp(0)=1 each to Z (subtracted as a constant) and are never
DMA'd out.  The one-hot (value K=128, keeps diag entries fp16-normal) is
built on the Vector engine via is_equal+mult in one op.  All pass-1 work
is emitted before pass-2 so the in-order PE queue never head-of-line
blocks, and exp/ln stay in one ACT table set.
"""

import sys

sys.path.insert(0, "/opt/trn_rl_repo")

import numpy as np

from concourse import bass, bacc, mybir
import concourse.tile as tile
from concourse.bass_utils import run_bass_kernel_spmd

NT, NS, BS, D, V = 128, 128, 8, 512, 32000
NCORES = 8
VS = V // NCORES  # 4000 vocab per core
VSP = 4096  # padded (bank-aligned) vocab per core
NPAIR = 4  # 1024-col (2 PSUM bank) pairs per batch
PW = VSP // NPAIR  # 1024
CH = 512  # cols per PSUM bank
P = 128
KC = D // P  # 4 contraction chunks
NG = 2  # Z-collective batch groups
GB = BS // NG  # batches per group
K = 128.0  # onehot scale: keeps diag entries in fp16 normal range
ESCALE = 64.0  # host emb prescale into fp8 range
HSCALE = 8.0  # host htgt prescale into fp8 range
F32 = mybir.dt.float32
F16 = mybir.dt.float16
F8 = mybir.dt.float8e4
I16 = mybir.dt.int16
AF = mybir.ActivationFunctionType
ALU = mybir.AluOpType
PM = mybir.MatmulPerfMode
INV_SQRT_D = 1.0 / float(np.sqrt(np.float32(D)))
NPAD = VSP - VS  # 96 pad columns -> exp(0)=1 each, subtracted from Z


def build_kernel():
    nc = bacc.Bacc(
        "TRN2",
        target_bir_lowering=False,
        debug=False,
        enable_asserts=False,
        num_devices=NCORES,
    )
    embT_h = nc.dram_tensor("embT_h", [KC, P, VSP], F8, kind="ExternalInput").ap()
    htgt8_h = nc.dram_tensor("htgt8_h", [P, KC, BS * P], F8, kind="ExternalInput").ap()
    htgt16_h = nc.dram_tensor(
        "htgt16_h", [KC, P, BS, P], F16, kind="ExternalInput"
    ).ap()
    hsrc16_h = nc.dram_tensor(
        "hsrc16_h", [KC, P, BS, P], F16, kind="ExternalInput"
    ).ap()
    qwT_h = nc.dram_tensor("qwT_h", [KC, P, D], F16, kind="ExternalInput").ap()
    qb_h = nc.dram_tensor("qb_h", [1, D], F16, kind="ExternalInput").ap()
    src_h = nc.dram_tensor("src_h", [NS, BS], F32, kind="ExternalInput").ap()
    w2_h = nc.dram_tensor("w2_h", [P, KC], F16, kind="ExternalInput").ap()
    nb2_h = nc.dram_tensor("nb2_h", [P, 1], F32, kind="ExternalInput").ap()
    iota_h = nc.dram_tensor("iota_h", [P, VSP], I16, kind="ExternalInput").ap()
    ident_h = nc.dram_tensor("ident_h", [P, P], F32, kind="ExternalInput").ap()
    out = nc.dram_tensor("out", [NT, BS, VS], F32, kind="ExternalOutput").ap()

    with tile.TileContext(nc) as tc:
        _emit(
            nc, tc, embT_h, htgt8_h, htgt16_h, hsrc16_h, qwT_h, qb_h, src_h,
            w2_h, nb2_h, iota_h, ident_h, out,
        )
    nc.compile()
    return nc


def _emit(
    nc, tc, embT_h, htgt8_h, htgt16_h, hsrc16_h, qwT_h, qb_h, src_h, w2_h,
    nb2_h, iota_h, ident_h, out,
):
    with (
        tc.tile_pool(name="persist", bufs=1) as pw,
        tc.tile_pool(name="small", bufs=2) as psm,
        tc.tile_pool(name="ps_gen", bufs=2, space="PSUM") as ps_gen,
        tc.tile_pool(name="ps_cp", bufs=2, space="PSUM") as ps_cp,
        tc.tile_pool(name="dram", bufs=1, space="DRAM") as pdram,
    ):
        dummy_in = pdram.tile([1, 64], F16, name="dummy_in")
        dummy_out = pdram.tile([NCORES, 64], F16, name="dummy_out")
        # absorb the cross-core rendezvous barrier as early as possible
        nc.gpsimd.collective_compute(
            "AllGather",
            ALU.bypass,
            replica_groups=[list(range(NCORES))],
            ins=[dummy_in[:].opt()],
            outs=[dummy_out[:].opt()],
        )

        # ---- persistent SBUF ----
        embT = pw.tile([P, KC, VSP], F8)  # (d, kc, v), x64
        htgt8 = pw.tile([P, KC, BS * P], F8)  # (d, kc, (b t)), x8
        htgt16 = pw.tile([P, KC, BS, P], F16)  # (d, kc, b, t)
        hsrc16 = pw.tile([P, KC, BS, P], F16)  # (d, kc, b, s)
        qwT = pw.tile([P, KC, D], F16)  # (d, kc, i)
        qb16 = pw.tile([1, D], F16)
        attnT_all = pw.tile([P, BS, NT], F16)  # (s, b, t)
        a_all = pw.tile([P, BS], F32)
        rc2K_all = pw.tile([P, BS], F32)  # K * (1-a) / a
        aK_all = pw.tile([P, BS], F32)  # a / K
        rc2KZ_all = pw.tile([P, BS], F32)  # K * (1-a) / (a*Z)
        src_sb = pw.tile([P, BS], F32)
        iota_all = pw.tile([P, VSP], I16)
        w2_sb = pw.tile([P, KC], F16)
        nb2_sb = pw.tile([P, 1], F32)
        identity = pw.tile([P, P], F32)
        ones16 = pw.tile([1, PW], F16)
        zparts = pw.tile([P, BS, NPAIR], F32)
        zloc = pw.tile([P, BS], F32)
        zg_sb = pw.tile([P, BS], F32)

        zin = [pdram.tile([P, GB], F32, name=f"zin{g}") for g in range(NG)]
        zout = [pdram.tile([P, GB], F32, name=f"zout{g}") for g in range(NG)]

        # ---- loads (small attention-critical ones first) ----
        nc.sync.dma_start(out=src_sb[:], in_=src_h)
        nc.sync.dma_start(out=w2_sb[:], in_=w2_h)
        nc.sync.dma_start(out=nb2_sb[:], in_=nb2_h)
        nc.sync.dma_start(out=qb16[:], in_=qb_h)
        nc.sync.dma_start(out=identity[:], in_=ident_h)
        for kc in range(KC):
            nc.sync.dma_start(out=qwT[:, kc], in_=qwT_h[kc])
        for kc in range(KC):
            nc.sync.dma_start(out=htgt16[:, kc], in_=htgt16_h[kc])
        for kc in range(KC):
            nc.sync.dma_start(out=hsrc16[:, kc], in_=hsrc16_h[kc])
        for kc in range(KC):
            nc.sync.dma_start(out=embT[:, kc], in_=embT_h[kc])
        nc.sync.dma_start(out=htgt8[:], in_=htgt8_h)
        nc.sync.dma_start(out=iota_all[:], in_=iota_h)
        nc.vector.memset(ones16[:], 1.0)

        def emit_attention_all(pat):
            qT_sb = pat.tile([P, KC, BS, P], F16)  # (i, ic, b, t)
            kT_sb = pat.tile([P, KC, BS, P], F16)  # (i, ic, b, s)
            attn_b = pat.tile([P, NS, 2], F32)  # (t, s, dbuf)
            # q/k projections for all batches at once (N=1024 matmuls)
            for dst, hT in ((qT_sb, htgt16), (kT_sb, hsrc16)):
                for ic in range(KC):
                    ps = ps_gen.tile([P, PW], F32, tag="g")
                    for h in range(2):  # PSUM bank limit: 512 f32 per matmul
                        col = slice(h * CH, (h + 1) * CH)
                        hb = slice(h * (BS // 2), (h + 1) * (BS // 2))
                        for kc in range(KC):
                            nc.tensor.matmul(
                                out=ps[:, col],
                                lhsT=qwT[:, kc, ic * P : (ic + 1) * P],
                                rhs=hT[:, kc, hb, :],
                                start=(kc == 0),
                                stop=False,
                            )
                        nc.tensor.matmul(
                            out=ps[:, col],
                            lhsT=qb16[:, ic * P : (ic + 1) * P],
                            rhs=ones16[:, 0:CH],
                            start=False,
                            stop=True,
                        )
                    nc.vector.tensor_copy(
                        out=dst[:, ic],
                        in_=ps[:].rearrange("i (b t) -> i b t", t=P),
                    )
            for b in range(BS):
                s_ps = ps_cp.tile([P, PW], F32, tag="c")
                for ic in range(KC):
                    nc.tensor.matmul(
                        out=s_ps[:, 0:P],
                        lhsT=qT_sb[:, ic, b, :],
                        rhs=kT_sb[:, ic, b, :],
                        start=(ic == 0),
                        stop=(ic == KC - 1),
                    )
                vw2_ps = ps_cp.tile([P, PW], F32, tag="c")
                for ic in range(KC):
                    nc.tensor.matmul(
                        out=vw2_ps[:, 0:1],
                        lhsT=kT_sb[:, ic, b, :],
                        rhs=w2_sb[:, ic : ic + 1],
                        start=(ic == 0),
                        stop=(ic == KC - 1),
                    )
                vw2_sb = psm.tile([P, 1], F16, tag="vw2")
                nc.vector.tensor_copy(out=vw2_sb[:], in_=vw2_ps[:, 0:1])
                m_col = psm.tile([P, 1], F32, tag="m")
                negm = psm.tile([P, 1], F32, tag="negm")
                zatt = psm.tile([P, 1], F32, tag="zatt")
                rz = psm.tile([P, 1], F32, tag="rz")
                ab = attn_b[:, :, b % 2]
                nc.vector.reduce_max(
                    out=m_col[:], in_=s_ps[:, 0:P], axis=mybir.AxisListType.X
                )
                nc.vector.tensor_scalar_mul(negm[:], m_col[:], -INV_SQRT_D)
                nc.scalar.activation(
                    out=ab,
                    in_=s_ps[:, 0:P],
                    func=AF.Exp,
                    bias=negm[:],
                    scale=INV_SQRT_D,
                    accum_out=zatt[:],
                )
                nc.vector.reciprocal(rz[:], zatt[:])
                nc.vector.tensor_scalar_mul(ab, ab, rz[:])
                t_ps = ps_cp.tile([P, PW], F32, tag="c")
                nc.tensor.transpose(t_ps[:, 0:P], ab, identity[:])
                nc.vector.tensor_copy(out=attnT_all[:, b, :], in_=t_ps[:, 0:P])
                # a = sigmoid(attn @ vw2 + b2), via exp (single ACT set)
                c_ps = ps_cp.tile([P, PW], F32, tag="c")
                nc.tensor.matmul(
                    out=c_ps[:, 0:1],
                    lhsT=attnT_all[:, b, :],
                    rhs=vw2_sb[:],
                    start=True,
                    stop=True,
                )
                ec = psm.tile([P, 1], F32, tag="ec")
                den = psm.tile([P, 1], F32, tag="den")
                nc.scalar.activation(
                    out=ec[:], in_=c_ps[:, 0:1], func=AF.Exp, scale=-1.0,
                    bias=nb2_sb[:],
                )
                nc.vector.tensor_scalar_add(den[:], ec[:], 1.0)
                nc.vector.reciprocal(a_all[:, b : b + 1], den[:])
            # K*(1-a)/a (Z applied later per group) and a/K for the Ln scale
            omaK = psm.tile([P, BS], F32, tag="omaK")
            ra = psm.tile([P, BS], F32, tag="ra")
            nc.vector.tensor_scalar(
                out=omaK[:], in0=a_all[:], scalar1=-K, scalar2=K,
                op0=ALU.mult, op1=ALU.add,
            )
            nc.vector.reciprocal(ra[:], a_all[:])
            nc.vector.tensor_tensor(
                out=rc2K_all[:], in0=omaK[:], in1=ra[:], op=ALU.mult
            )
            nc.vector.tensor_scalar_mul(aK_all[:], a_all[:], 1.0 / K)

        with tc.tile_pool(name="attn", bufs=1) as pat:
            emit_attention_all(pat)

        with (
            tc.tile_pool(name="e", bufs=BS) as pe,
            tc.tile_pool(name="oh", bufs=BS) as poh,
            tc.tile_pool(name="dg", bufs=BS) as pdg,
            tc.tile_pool(name="ot", bufs=3) as pot,
        ):
            e_tiles = {}

            def emit_pass1_batch(b):
                e_tiles[b] = pe.tile([P, VSP], F16, tag="e", name=f"e_{b}")
                for p in range(NPAIR):
                    g_ps = ps_gen.tile([P, PW], F32, tag="g")
                    for h in range(2):
                        col = slice(h * CH, (h + 1) * CH)
                        vcol = slice(p * PW + h * CH, p * PW + (h + 1) * CH)
                        for kh in (0, 2):  # kc pairs, fp8 DoubleRow
                            nc.tensor.matmul(
                                out=g_ps[:, col],
                                lhsT=htgt8[:, kh : kh + 2, b * P : (b + 1) * P],
                                rhs=embT[:, kh : kh + 2, vcol],
                                start=(kh == 0),
                                stop=(kh == 2),
                                perf_mode=PM.DoubleRow,
                            )
                    nc.scalar.activation(
                        out=e_tiles[b][:, p * PW : (p + 1) * PW],
                        in_=g_ps[:],
                        func=AF.Exp,
                        scale=1.0 / (ESCALE * HSCALE),
                        accum_out=zparts[:, b, p : p + 1],
                    )

            def emit_group_z(g):
                gs = slice(g * GB, (g + 1) * GB)
                nc.vector.reduce_sum(
                    out=zloc[:, gs], in_=zparts[:, gs, :], axis=mybir.AxisListType.X
                )
                # remove the VSP-VS zero-pad columns' exp(0)=1 contributions
                nc.vector.tensor_scalar_add(zloc[:, gs], zloc[:, gs], -float(NPAD))
                nc.sync.dma_start(out=zin[g][:], in_=zloc[:, gs])
                nc.gpsimd.collective_compute(
                    "AllReduce",
                    ALU.add,
                    replica_groups=[list(range(NCORES))],
                    ins=[zin[g][:].opt()],
                    outs=[zout[g][:].opt()],
                )
                nc.sync.dma_start(out=zg_sb[:, gs], in_=zout[g][:])

            def emit_group_coefs(g, diags):
                # diag scale K*(1-a)/(a*Z); built on DVE (~450ns each) so the
                # Scalar queue never gates on the AllReduce
                gs = slice(g * GB, (g + 1) * GB)
                rzg = psm.tile([P, GB], F32, tag="rzg")
                nc.vector.reciprocal(rzg[:], zg_sb[:, gs])
                nc.vector.tensor_tensor(
                    out=rc2KZ_all[:, gs], in0=rc2K_all[:, gs], in1=rzg[:],
                    op=ALU.mult,
                )
                for b in range(g * GB, (g + 1) * GB):
                    d = pdg.tile([P, P], F16, tag="dg", name=f"dg_{b}")
                    nc.vector.tensor_scalar_mul(
                        d[:], identity[:], rc2KZ_all[:, b : b + 1]
                    )
                    diags[b] = d

            def emit_onehot(b):
                onehot = poh.tile([P, VSP], F16, tag="oh", name=f"oh_{b}")
                nc.vector.tensor_scalar(
                    out=onehot[:],
                    in0=iota_all[:],
                    scalar1=src_sb[:, b : b + 1],
                    scalar2=K,
                    op0=ALU.is_equal,
                    op1=ALU.mult,
                )
                return onehot

            def emit_pass2_batch(b, diag, onehot):
                e_b = e_tiles[b]
                for p in range(NPAIR):
                    cp_ps = ps_cp.tile([P, PW], F32, tag="c")
                    for h in range(2):
                        col = slice(h * CH, (h + 1) * CH)
                        vcol = slice(p * PW + h * CH, p * PW + (h + 1) * CH)
                        nc.tensor.matmul(
                            out=cp_ps[:, col],
                            lhsT=attnT_all[:, b, :],
                            rhs=onehot[:, vcol],
                            start=True,
                            stop=False,
                        )
                        nc.tensor.matmul(
                            out=cp_ps[:, col],
                            lhsT=diag[:],
                            rhs=e_b[:, vcol],
                            start=False,
                            stop=True,
                        )
                    outt = pot.tile([P, PW], F32, tag="ot")
                    nc.scalar.activation(
                        out=outt[:],
                        in_=cp_ps[:],
                        func=AF.Ln,
                        scale=aK_all[:, b : b + 1],
                    )
                    w = min(VS - p * PW, PW)
                    nc.sync.dma_start(
                        out=out[:, b, p * PW : p * PW + w], in_=outt[:, 0:w]
                    )

            diags = [None] * BS
            onehots = [emit_onehot(b) for b in range(BS)]
            for b in range(GB):
                emit_pass1_batch(b)
            emit_group_z(0)
            emit_group_coefs(0, diags)
            for b in range(GB, BS):
                emit_pass1_batch(b)
            emit_group_z(1)
            emit_group_coefs(1, diags)
            for b in range(BS):
                emit_pass2_batch(b, diags[b], onehots[b])


_NC_CACHE = []


def _get_nc():
    if not _NC_CACHE:
        _NC_CACHE.append(build_kernel())
    return _NC_CACHE[0]


def _make_in_maps(inputs):
    htgt = np.asarray(inputs["htgt"], dtype=np.float32)
    hsrc = np.asarray(inputs["hsrc"], dtype=np.float32)
    src = np.asarray(inputs["src"]).astype(np.int64)
    emb = np.asarray(inputs["emb_weight"], dtype=np.float32)
    q_w = np.asarray(inputs["q_w"], dtype=np.float32)
    q_b = np.asarray(inputs["q_b"], dtype=np.float32)
    f_w = np.asarray(inputs["f_w"], dtype=np.float32)
    f_b = np.asarray(inputs["f_b"], dtype=np.float32)
    copy_w = np.asarray(inputs["copy_w"], dtype=np.float32)
    copy_b = np.asarray(inputs["copy_b"], dtype=np.float32)
    np8 = mybir.dt.np(F8)

    # shared across cores
    htgtT = htgt.transpose(2, 1, 0)  # (d, b, t)
    htgt16_h = np.ascontiguousarray(htgtT.astype(np.float16).reshape(KC, P, BS, P))
    hsrc16_h = np.ascontiguousarray(
        hsrc.transpose(2, 1, 0).astype(np.float16).reshape(KC, P, BS, P)
    )
    # fp8 gen copy: (d_inner, kc, (b t)), scaled by HSCALE
    htgt8_h = np.ascontiguousarray(
        (htgtT * HSCALE)
        .reshape(KC, P, BS * P)
        .transpose(1, 0, 2)
        .astype(np8)
    )
    qwT_h = np.ascontiguousarray(q_w.T).astype(np.float16).reshape(KC, P, D)
    qb_h = np.ascontiguousarray(q_b.astype(np.float16).reshape(1, D))
    # fold f_w/copy_w:  w2 = f_w.T @ copy_w.T ;  b2 = copy_w @ f_b + copy_b
    w2_full = f_w.T @ copy_w[0]  # [D]
    w2_h = np.ascontiguousarray(w2_full.reshape(KC, P).T.astype(np.float16))
    nb2 = -(copy_w[0] @ f_b + copy_b[0])
    nb2_h = np.ascontiguousarray(np.full((P, 1), nb2, dtype=np.float32))
    iota_h = np.ascontiguousarray(
        np.broadcast_to(np.arange(VSP, dtype=np.int16), (P, VSP))
    )
    ident_h = np.ascontiguousarray(np.eye(P, dtype=np.float32))

    in_maps = []
    for c in range(NCORES):
        eT = np.zeros((D, VSP), dtype=np8)
        eT[:, 0:VS] = (emb[c * VS : (c + 1) * VS].T * ESCALE).astype(np8)
        embT_h = np.ascontiguousarray(eT.reshape(KC, P, VSP))
        # integral values, exact in fp32 (scalar operand of is_equal is f32)
        src_local = np.ascontiguousarray((src - c * VS).astype(np.float32))
        in_maps.append(
            {
                "embT_h": embT_h,
                "htgt8_h": htgt8_h,
                "htgt16_h": htgt16_h,
                "hsrc16_h": hsrc16_h,
                "qwT_h": qwT_h,
                "qb_h": qb_h,
                "src_h": src_local,
                "w2_h": w2_h,
                "nb2_h": nb2_h,
                "iota_h": iota_h,
                "ident_h": ident_h,
            }
        )
    return in_maps


def kernel(**inputs):
    in_maps = _make_in_maps(inputs)
    nc = _get_nc()
    res = run_bass_kernel_spmd(nc, in_maps, list(range(NCORES))).results
    return np.concatenate([res[c]["out"] for c in range(NCORES)], axis=2)


# revision 11
# speedup vs baseline: 1.1669x; 1.1669x over previous
"""CopyGenerator kernel for Trainium2 (Bass/Tile), vocab-parallel over 8 cores.

Per core c (vocab shard [c*4000, (c+1)*4000)):
  A tiny dummy AllGather fires at t=0 to absorb the cross-core rendezvous
  barrier while compute proceeds.
  Attention for ALL 8 batches is computed locally (redundantly) on every
  core — cheaper than an AllGather whose latency floor + launch skew was
  ~60us.  q/k projections are batched (N=512 matmuls); the softmax skips
  max-subtraction (logits are O(1)) so a single exp covers all batches,
  and the vw2 = k @ w2 reduction is host-folded to u = q_w.T @ w2 so the
  Scalar engine runs only two ACT ops in the whole attention phase.
  pass 1: gen = htgt @ emb_shard.T  (PE, fp8 DoubleRow: emb pre-scaled by
    64 and htgt by 8 on the host into fp8 normal range; exp's free scale
    1/512 undoes it).  e = exp(gen/512) over 2048-wide PSUM quads with
    fused row-sum accumulation.  Z = allreduce_add(rowsum), two batch
    groups, the first AR overlaps the second half of pass 1.
  pass 2: psum = K*copy_p + diag(K*(1-a)/(a*Z)) @ e, all on the PE
    (attnT @ K*onehot(src) accumulated with the diag matmul), then
    out = Ln((a/K) * psum) straight from PSUM, 2048 cols per ACT.
      check: (a/K)*(K*cp + K*(1-a)/(a*Z)*e) = a*cp + (1-a)*e/Z. ✓
  The diag matrices are built on the Vector engine so the Scalar queue
  never gates on the AllReduce; K=128 keeps diag entries fp16-normal.

All transposed operands are pre-transposed/cast on the host and DMA
straight into SBUF (no PE transposes except the 128x128 attn ones).
The vocab shard is zero-padded to 4096 for PSUM bank alignment; pad
columns add exp(0)=1 each to Z (subtracted as a constant) and are never
DMA'd out.  All pass-1 work is emitted before pass-2 so the in-order PE
queue never head-of-line blocks, and exp/ln stay in one ACT table set.
"""

import sys

sys.path.insert(0, "/opt/trn_rl_repo")

import numpy as np

from concourse import bass, bacc, mybir
import concourse.tile as tile
from concourse.bass_utils import run_bass_kernel_spmd

NT, NS, BS, D, V = 128, 128, 8, 512, 32000
NCORES = 8
VS = V // NCORES  # 4000 vocab per core
VSP = 4096  # padded (bank-aligned) vocab per core
NQ = 2  # 2048-col (4 PSUM bank) quads per batch
QW = VSP // NQ  # 2048
CH = 512  # cols per PSUM bank
P = 128
KC = D // P  # 4 contraction chunks
NG = 2  # Z-collective batch groups
GB = BS // NG  # batches per group
K = 128.0  # onehot scale: keeps diag entries in fp16 normal range
ESCALE = 64.0  # host emb prescale into fp8 range
HSCALE = 8.0  # host htgt prescale into fp8 range
F32 = mybir.dt.float32
F16 = mybir.dt.float16
F8 = mybir.dt.float8e4
I16 = mybir.dt.int16
AF = mybir.ActivationFunctionType
ALU = mybir.AluOpType
PM = mybir.MatmulPerfMode
INV_SQRT_D = 1.0 / float(np.sqrt(np.float32(D)))
NPAD = VSP - VS  # 96 pad columns -> exp(0)=1 each, subtracted from Z


def build_kernel():
    nc = bacc.Bacc(
        "TRN2",
        target_bir_lowering=False,
        debug=False,
        enable_asserts=False,
        num_devices=NCORES,
    )
    embT_h = nc.dram_tensor("embT_h", [KC, P, VSP], F8, kind="ExternalInput").ap()
    htgt8_h = nc.dram_tensor("htgt8_h", [P, KC, BS * P], F8, kind="ExternalInput").ap()
    htgt16_h = nc.dram_tensor(
        "htgt16_h", [KC, P, BS, P], F16, kind="ExternalInput"
    ).ap()
    hsrc16_h = nc.dram_tensor(
        "hsrc16_h", [KC, P, BS, P], F16, kind="ExternalInput"
    ).ap()
    qwT_h = nc.dram_tensor("qwT_h", [KC, P, D], F16, kind="ExternalInput").ap()
    qb_h = nc.dram_tensor("qb_h", [1, D], F16, kind="ExternalInput").ap()
    src_h = nc.dram_tensor("src_h", [NS, BS], F32, kind="ExternalInput").ap()
    u_h = nc.dram_tensor("u_h", [P, KC], F16, kind="ExternalInput").ap()
    nb2_h = nc.dram_tensor("nb2_h", [P, 1], F32, kind="ExternalInput").ap()
    iota_h = nc.dram_tensor("iota_h", [P, VSP], I16, kind="ExternalInput").ap()
    ident_h = nc.dram_tensor("ident_h", [P, P], F32, kind="ExternalInput").ap()
    out = nc.dram_tensor("out", [NT, BS, VS], F32, kind="ExternalOutput").ap()

    with tile.TileContext(nc) as tc:
        _emit(
            nc, tc, embT_h, htgt8_h, htgt16_h, hsrc16_h, qwT_h, qb_h, src_h,
            u_h, nb2_h, iota_h, ident_h, out,
        )
    nc.compile()
    return nc


def _emit(
    nc, tc, embT_h, htgt8_h, htgt16_h, hsrc16_h, qwT_h, qb_h, src_h, u_h,
    nb2_h, iota_h, ident_h, out,
):
    with (
        tc.tile_pool(name="persist", bufs=1) as pw,
        tc.tile_pool(name="small", bufs=2) as psm,
        tc.tile_pool(name="ps", bufs=2, space="PSUM") as ps,
        tc.tile_pool(name="dram", bufs=1, space="DRAM") as pdram,
    ):
        dummy_in = pdram.tile([1, 64], F16, name="dummy_in")
        dummy_out = pdram.tile([NCORES, 64], F16, name="dummy_out")
        # absorb the cross-core rendezvous barrier as early as possible
        nc.gpsimd.collective_compute(
            "AllGather",
            ALU.bypass,
            replica_groups=[list(range(NCORES))],
            ins=[dummy_in[:].opt()],
            outs=[dummy_out[:].opt()],
        )

        # ---- persistent SBUF ----
        embT = pw.tile([P, KC, VSP], F8)  # (d, kc, v), x64
        htgt8 = pw.tile([P, KC, BS * P], F8)  # (d, kc, (b t)), x8
        qwT = pw.tile([P, KC, D], F16)  # (d, kc, i)
        qb16 = pw.tile([1, D], F16)
        attnT_all = pw.tile([P, BS, NT], F16)  # (s, b, t)
        a_all = pw.tile([P, BS], F32)
        rc2K_all = pw.tile([P, BS], F32)  # K * (1-a) / a
        aK_all = pw.tile([P, BS], F32)  # a / K
        rc2KZ_all = pw.tile([P, BS], F32)  # K * (1-a) / (a*Z)
        src_sb = pw.tile([P, BS], F32)
        iota_all = pw.tile([P, VSP], I16)
        u_sb = pw.tile([P, KC], F16)
        nb2_sb = pw.tile([P, 1], F32)
        identity = pw.tile([P, P], F32)
        ones16 = pw.tile([1, CH], F16)
        zparts = pw.tile([P, BS, NQ], F32)
        zloc = pw.tile([P, BS], F32)
        zg_sb = pw.tile([P, BS], F32)

        zin = [pdram.tile([P, GB], F32, name=f"zin{g}") for g in range(NG)]
        zout = [pdram.tile([P, GB], F32, name=f"zout{g}") for g in range(NG)]

        # ---- loads (small attention-critical ones first) ----
        nc.sync.dma_start(out=src_sb[:], in_=src_h)
        nc.sync.dma_start(out=u_sb[:], in_=u_h)
        nc.sync.dma_start(out=nb2_sb[:], in_=nb2_h)
        nc.sync.dma_start(out=qb16[:], in_=qb_h)
        nc.sync.dma_start(out=identity[:], in_=ident_h)
        for kc in range(KC):
            nc.sync.dma_start(out=qwT[:, kc], in_=qwT_h[kc])
        nc.vector.memset(ones16[:], 1.0)

        def emit_attention_all(pat):
            htgt16 = pat.tile([P, KC, BS, P], F16)  # (d, kc, b, t)
            hsrc16 = pat.tile([P, KC, BS, P], F16)  # (d, kc, b, s)
            for kc in range(KC):
                nc.sync.dma_start(out=htgt16[:, kc], in_=htgt16_h[kc])
            for kc in range(KC):
                nc.sync.dma_start(out=hsrc16[:, kc], in_=hsrc16_h[kc])

            qT_sb = pat.tile([P, KC, BS, P], F16)  # (i, ic, b, t)
            kT_sb = pat.tile([P, KC, BS, P], F16)  # (i, ic, b, s)
            attn_all = pat.tile([P, BS, NS], F32)  # (t, b, s)
            srow = psm.tile([P, BS], F32, tag="srow")
            rz_all = psm.tile([P, BS], F32, tag="rz")
            vw2_all = psm.tile([P, BS], F16, tag="vw2")
            ec_all = psm.tile([P, BS], F32, tag="ec")
            den_all = psm.tile([P, BS], F32, tag="den")

            # q/k projections, batched (PSUM bank limit: 512 f32 per matmul)
            for dst, hT in ((qT_sb, htgt16), (kT_sb, hsrc16)):
                for ic in range(KC):
                    p_qk = ps.tile([P, QW], F32, tag="g")
                    for h in range(2):
                        col = slice(h * CH, (h + 1) * CH)
                        hb = slice(h * (BS // 2), (h + 1) * (BS // 2))
                        for kc in range(KC):
                            nc.tensor.matmul(
                                out=p_qk[:, col],
                                lhsT=qwT[:, kc, ic * P : (ic + 1) * P],
                                rhs=hT[:, kc, hb, :],
                                start=(kc == 0),
                                stop=False,
                            )
                        nc.tensor.matmul(
                            out=p_qk[:, col],
                            lhsT=qb16[:, ic * P : (ic + 1) * P],
                            rhs=ones16[:],
                            start=False,
                            stop=True,
                        )
                    nc.vector.tensor_copy(
                        out=dst[:, ic],
                        in_=p_qk[:, 0 : BS * P].rearrange("i (b t) -> i b t", t=P),
                    )

            # scores for all batches into one slot; softmax without max-sub
            # (logits are O(1)); one exp covers every batch
            s_slot = ps.tile([P, QW], F32, tag="g")
            for b in range(BS):
                for ic in range(KC):
                    nc.tensor.matmul(
                        out=s_slot[:, b * P : (b + 1) * P],
                        lhsT=qT_sb[:, ic, b, :],
                        rhs=kT_sb[:, ic, b, :],
                        start=(ic == 0),
                        stop=(ic == KC - 1),
                    )
            nc.scalar.activation(
                out=attn_all[:],
                in_=s_slot[:, 0 : BS * NS],
                func=AF.Exp,
                scale=INV_SQRT_D,
            )
            nc.vector.reduce_sum(
                out=srow[:], in_=attn_all[:], axis=mybir.AxisListType.X
            )
            nc.vector.reciprocal(rz_all[:], srow[:])
            for b in range(BS):
                nc.vector.tensor_scalar_mul(
                    attn_all[:, b, :], attn_all[:, b, :], rz_all[:, b : b + 1]
                )
            t_slot = ps.tile([P, QW], F32, tag="g")
            for b in range(BS):
                nc.tensor.transpose(
                    t_slot[:, b * P : (b + 1) * P], attn_all[:, b, :], identity[:]
                )
            nc.vector.tensor_copy(
                out=attnT_all[:].rearrange("s b t -> s (b t)"),
                in_=t_slot[:, 0 : BS * NT],
            )
            # vw2 = k @ w2 host-folded to hsrc @ u (u = q_w.T @ w2; the qb
            # part is constant since attn rows sum to 1 -> folded into nb2)
            vw2_ps = ps.tile([P, QW], F32, tag="g")
            for b in range(BS):
                for kc in range(KC):
                    nc.tensor.matmul(
                        out=vw2_ps[:, b : b + 1],
                        lhsT=hsrc16[:, kc, b, :],
                        rhs=u_sb[:, kc : kc + 1],
                        start=(kc == 0),
                        stop=(kc == KC - 1),
                    )
            nc.vector.tensor_copy(out=vw2_all[:], in_=vw2_ps[:, 0:BS])
            c_slot = ps.tile([P, QW], F32, tag="g")
            for b in range(BS):
                nc.tensor.matmul(
                    out=c_slot[:, b : b + 1],
                    lhsT=attnT_all[:, b, :],
                    rhs=vw2_all[:, b : b + 1],
                    start=True,
                    stop=True,
                )
            # a = sigmoid(c + b2) via exp (single ACT table set)
            nc.scalar.activation(
                out=ec_all[:], in_=c_slot[:, 0:BS], func=AF.Exp, scale=-1.0,
                bias=nb2_sb[:],
            )
            nc.vector.tensor_scalar_add(den_all[:], ec_all[:], 1.0)
            nc.vector.reciprocal(a_all[:], den_all[:])
            # K*(1-a)/a (Z applied later per group) and a/K for the Ln scale
            omaK = psm.tile([P, BS], F32, tag="omaK")
            ra = psm.tile([P, BS], F32, tag="ra")
            nc.vector.tensor_scalar(
                out=omaK[:], in0=a_all[:], scalar1=-K, scalar2=K,
                op0=ALU.mult, op1=ALU.add,
            )
            nc.vector.reciprocal(ra[:], a_all[:])
            nc.vector.tensor_tensor(
                out=rc2K_all[:], in0=omaK[:], in1=ra[:], op=ALU.mult
            )
            nc.vector.tensor_scalar_mul(aK_all[:], a_all[:], 1.0 / K)

        with tc.tile_pool(name="attn", bufs=1) as pat:
            emit_attention_all(pat)

        with (
            tc.tile_pool(name="e", bufs=BS) as pe,
            tc.tile_pool(name="oh", bufs=BS) as poh,
            tc.tile_pool(name="dg", bufs=BS) as pdg,
            tc.tile_pool(name="ot", bufs=2) as pot,
        ):
            e_tiles = {}

            def emit_onehot(b):
                onehot = poh.tile([P, VSP], F16, tag="oh", name=f"oh_{b}")
                nc.vector.tensor_scalar(
                    out=onehot[:],
                    in0=iota_all[:],
                    scalar1=src_sb[:, b : b + 1],
                    scalar2=K,
                    op0=ALU.is_equal,
                    op1=ALU.mult,
                )
                return onehot

            def emit_pass1_batch(b):
                e_tiles[b] = pe.tile([P, VSP], F16, tag="e", name=f"e_{b}")
                for q in range(NQ):
                    g_ps = ps.tile([P, QW], F32, tag="g")
                    for h in range(4):
                        col = slice(h * CH, (h + 1) * CH)
                        vcol = slice(q * QW + h * CH, q * QW + (h + 1) * CH)
                        for kh in (0, 2):  # kc pairs, fp8 DoubleRow
                            nc.tensor.matmul(
                                out=g_ps[:, col],
                                lhsT=htgt8[:, kh : kh + 2, b * P : (b + 1) * P],
                                rhs=embT[:, kh : kh + 2, vcol],
                                start=(kh == 0),
                                stop=(kh == 2),
                                perf_mode=PM.DoubleRow,
                            )
                    nc.scalar.activation(
                        out=e_tiles[b][:, q * QW : (q + 1) * QW],
                        in_=g_ps[:],
                        func=AF.Exp,
                        scale=1.0 / (ESCALE * HSCALE),
                        accum_out=zparts[:, b, q : q + 1],
                    )

            def emit_group_z(g):
                gs = slice(g * GB, (g + 1) * GB)
                nc.vector.reduce_sum(
                    out=zloc[:, gs], in_=zparts[:, gs, :], axis=mybir.AxisListType.X
                )
                # remove the VSP-VS zero-pad columns' exp(0)=1 contributions
                nc.vector.tensor_scalar_add(zloc[:, gs], zloc[:, gs], -float(NPAD))
                nc.sync.dma_start(out=zin[g][:], in_=zloc[:, gs])
                nc.gpsimd.collective_compute(
                    "AllReduce",
                    ALU.add,
                    replica_groups=[list(range(NCORES))],
                    ins=[zin[g][:].opt()],
                    outs=[zout[g][:].opt()],
                )
                nc.sync.dma_start(out=zg_sb[:, gs], in_=zout[g][:])

            def emit_group_coefs(g, diags):
                # diag scale K*(1-a)/(a*Z); built on DVE (~450ns each) so the
                # Scalar queue never gates on the AllReduce
                gs = slice(g * GB, (g + 1) * GB)
                rzg = psm.tile([P, GB], F32, tag="rzg")
                nc.vector.reciprocal(rzg[:], zg_sb[:, gs])
                nc.vector.tensor_tensor(
                    out=rc2KZ_all[:, gs], in0=rc2K_all[:, gs], in1=rzg[:],
                    op=ALU.mult,
                )
                for b in range(g * GB, (g + 1) * GB):
                    d = pdg.tile([P, P], F16, tag="dg", name=f"dg_{b}")
                    nc.vector.tensor_scalar_mul(
                        d[:], identity[:], rc2KZ_all[:, b : b + 1]
                    )
                    diags[b] = d

            def emit_pass2_batch(b, diag, onehot):
                e_b = e_tiles[b]
                for q in range(NQ):
                    cp_ps = ps.tile([P, QW], F32, tag="g")
                    for h in range(4):
                        col = slice(h * CH, (h + 1) * CH)
                        vcol = slice(q * QW + h * CH, q * QW + (h + 1) * CH)
                        nc.tensor.matmul(
                            out=cp_ps[:, col],
                            lhsT=attnT_all[:, b, :],
                            rhs=onehot[:, vcol],
                            start=True,
                            stop=False,
                        )
                        nc.tensor.matmul(
                            out=cp_ps[:, col],
                            lhsT=diag[:],
                            rhs=e_b[:, vcol],
                            start=False,
                            stop=True,
                        )
                    outt = pot.tile([P, QW], F32, tag="ot")
                    nc.scalar.activation(
                        out=outt[:],
                        in_=cp_ps[:],
                        func=AF.Ln,
                        scale=aK_all[:, b : b + 1],
                    )
                    w = min(VS - q * QW, QW)
                    nc.sync.dma_start(
                        out=out[:, b, q * QW : q * QW + w], in_=outt[:, 0:w]
                    )

            nc.sync.dma_start(out=iota_all[:], in_=iota_h)
            for kc in range(KC):
                nc.sync.dma_start(out=embT[:, kc], in_=embT_h[kc])
            nc.sync.dma_start(out=htgt8[:], in_=htgt8_h)

            diags = [None] * BS
            onehots = [emit_onehot(b) for b in range(BS)]
            for b in range(GB):
                emit_pass1_batch(b)
            emit_group_z(0)
            emit_group_coefs(0, diags)
            for b in range(GB, BS):
                emit_pass1_batch(b)
            emit_group_z(1)
            emit_group_coefs(1, diags)
            for b in range(BS):
                emit_pass2_batch(b, diags[b], onehots[b])


_NC_CACHE = []


def _get_nc():
    if not _NC_CACHE:
        _NC_CACHE.append(build_kernel())
    return _NC_CACHE[0]


def _make_in_maps(inputs):
    htgt = np.asarray(inputs["htgt"], dtype=np.float32)
    hsrc = np.asarray(inputs["hsrc"], dtype=np.float32)
    src = np.asarray(inputs["src"]).astype(np.int64)
    emb = np.asarray(inputs["emb_weight"], dtype=np.float32)
    q_w = np.asarray(inputs["q_w"], dtype=np.float32)
    q_b = np.asarray(inputs["q_b"], dtype=np.float32)
    f_w = np.asarray(inputs["f_w"], dtype=np.float32)
    f_b = np.asarray(inputs["f_b"], dtype=np.float32)
    copy_w = np.asarray(inputs["copy_w"], dtype=np.float32)
    copy_b = np.asarray(inputs["copy_b"], dtype=np.float32)
    np8 = mybir.dt.np(F8)

    # shared across cores
    htgtT = htgt.transpose(2, 1, 0)  # (d, b, t)
    htgt16_h = np.ascontiguousarray(htgtT.astype(np.float16).reshape(KC, P, BS, P))
    hsrc16_h = np.ascontiguousarray(
        hsrc.transpose(2, 1, 0).astype(np.float16).reshape(KC, P, BS, P)
    )
    # fp8 gen copy: (d_inner, kc, (b t)), scaled by HSCALE
    htgt8_h = np.ascontiguousarray(
        (htgtT * HSCALE).reshape(KC, P, BS * P).transpose(1, 0, 2).astype(np8)
    )
    qwT_h = np.ascontiguousarray(q_w.T).astype(np.float16).reshape(KC, P, D)
    qb_h = np.ascontiguousarray(q_b.astype(np.float16).reshape(1, D))
    # fold f_w/copy_w: w2 = f_w.T @ copy_w.T; u = q_w.T @ w2;
    # b2' = copy_w @ f_b + copy_b + q_b @ w2 (attn rows sum to 1)
    w2_full = f_w.T @ copy_w[0]  # [D]
    u_full = q_w.T @ w2_full  # [D]
    u_h = np.ascontiguousarray(u_full.reshape(KC, P).T.astype(np.float16))
    nb2 = -(copy_w[0] @ f_b + copy_b[0] + q_b @ w2_full)
    nb2_h = np.ascontiguousarray(np.full((P, 1), nb2, dtype=np.float32))
    iota_h = np.ascontiguousarray(
        np.broadcast_to(np.arange(VSP, dtype=np.int16), (P, VSP))
    )
    ident_h = np.ascontiguousarray(np.eye(P, dtype=np.float32))

    in_maps = []
    for c in range(NCORES):
        eT = np.zeros((D, VSP), dtype=np8)
        eT[:, 0:VS] = (emb[c * VS : (c + 1) * VS].T * ESCALE).astype(np8)
        embT_h = np.ascontiguousarray(eT.reshape(KC, P, VSP))
        # integral values, exact in fp32 (scalar operand of is_equal is f32)
        src_local = np.ascontiguousarray((src - c * VS).astype(np.float32))
        in_maps.append(
            {
                "embT_h": embT_h,
                "htgt8_h": htgt8_h,
                "htgt16_h": htgt16_h,
                "hsrc16_h": hsrc16_h,
                "qwT_h": qwT_h,
                "qb_h": qb_h,
                "src_h": src_local,
                "u_h": u_h,
                "nb2_h": nb2_h,
                "iota_h": iota_h,
                "ident_h": ident_h,
            }
        )
    return in_maps


def kernel(**inputs):
    in_maps = _make_in_maps(inputs)
    nc = _get_nc()
    res = run_bass_kernel_spmd(nc, in_maps, list(range(NCORES))).results
    return np.concatenate([res[c]["out"] for c in range(NCORES)], axis=2)
